# revision 28
# baseline (speedup 1.0000x reference)
"""DecoupledCrossAttention Trainium2 kernel (8 NeuronCores, Bass/Tile).

Reference computation (per batch b of 4, DIM=512, 8 heads x 64):
    q = heads(x @ Wq.T + bq)
    x_audio  = attn(q, audio_context;  Wka, bka, Wva, bva)   # m=2048
    x_singer = attn(q, singer_context; Wks, bks, Wvs, bvs)   # m=256
    out = (x_audio + x_singer) @ Wp.T + bp

Sharding: 8 cores = 4 batches x 2 head-groups (4 heads = 256 feat each).
Each core computes its batch/head-group attention and a PARTIAL output
projection (its 256-dim slice of the Wp contraction); the host sums the
two partials per batch and adds bp.

Single flat pipeline (no phase-scoped pools -- pool releases create
soft barriers through PSUM address reuse). The scalar (ACT) engine's
exp stream is the binding resource (~1us per [128,1024] tile, ~144us
total), so everything else is arranged to keep it gapless:
  - PSUM plan (8 banks): sA 2x[128,1024] (4) reserved for QK scores /
    out-proj; pv 1x[65,1024] (2); psW 2x[128,512] (2) for ALL
    projection accumulators so they never steal an sA slot mid-stream.
  - QK uses zero-padded q halves against the plain kT pair tile
    (rows 0:64 = even head, 64:128 = odd head) -- no block-diagonal k
    build, no q duplication DMAs.
  - v is projected directly in natural [token, dim] layout (stationary
    = ctx tile, moving = Wv chunk) -- no PE transposes, no identity.
  - attends are split over query halves (nh) so pv needs only 2 banks.
  - softmax denominator row leaves PSUM by DMA (not DVE); reciprocal
    on DVE; partition broadcast on GpSimd; combines on DVE.
  - projections for later heads / singer / out-proj are emitted after
    attend(h0) so the scheduler drops them into PE slack under the
    exp stream; out_t is fp16 (host sums partials in fp32).
"""
import numpy as np
import ml_dtypes
from contextlib import ExitStack

import concourse.bass as bass
import concourse.tile as tile
from concourse import bacc, mybir
from concourse import bass_utils

F32 = mybir.dt.float32
F16 = mybir.dt.float16
F32R = mybir.dt.bfloat16  # matmul operand dtype (bf16)
F8 = mybir.dt.float8e4   # e4m3: exp weights + v for the PV DoubleRow matmul
AF = mybir.ActivationFunctionType
OP = mybir.AluOpType

DIM = 512
HS = 256             # feature slice per core (4 heads x 64)
HD = 64              # head dim
N = 2048             # query tokens
MA = 2048            # audio context tokens
MS = 256             # singer context tokens
B = 4
SCALE = float(DIM) ** -0.5
MMN = 1024
VPAD = 80           # v-natural per-head stride (16B-aligned for fp8 DoubleRow LDW)


def _build():
    nc = bacc.Bacc("TRN2", target_bir_lowering=False, debug=False,
                   enable_asserts=True, num_devices=8)

    def din(name, shape, dt=F32R):
        return nc.dram_tensor(name, shape, dt, kind="ExternalInput").ap()

    xT = din("xT", [DIM, N])
    caT = din("caT", [DIM, MA])
    csT = din("csT", [DIM, MS])
    wqT = din("wqT", [DIM, HS])
    wkaT = din("wkaT", [DIM, HS])
    wvaT = din("wvaT", [DIM, HS])
    wksT = din("wksT", [DIM, HS])
    wvsT = din("wvsT", [DIM, HS])
    wpT = din("wpT", [HS, DIM])
    bq = din("bq", [HS], F32)
    bka = din("bka", [HS], F32)
    bks = din("bks", [HS], F32)
    bvv = din("bvv", [HS], F32)
    out_a = nc.dram_tensor("out_a", [DIM, N], F16, kind="ExternalOutput").ap()
    out_b = nc.dram_tensor("out_b", [DIM, N], F16, kind="ExternalOutput").ap()

    with tile.TileContext(nc) as tc, ExitStack() as ctx:
        const = ctx.enter_context(tc.tile_pool(name="const", bufs=1))
        wpool = ctx.enter_context(tc.tile_pool(name="wpool", bufs=1))
        ctxp = ctx.enter_context(tc.tile_pool(name="ctxp", bufs=1))
        actp = ctx.enter_context(tc.tile_pool(name="actp", bufs=1))
        ep = ctx.enter_context(tc.tile_pool(name="ep", bufs=10))
        up = ctx.enter_context(tc.tile_pool(name="up", bufs=3))
        rpool = ctx.enter_context(tc.tile_pool(name="rpool", bufs=4))
        ostage = ctx.enter_context(tc.tile_pool(name="ostage", bufs=4))
        psS = ctx.enter_context(tc.tile_pool(name="psS", bufs=2, space="PSUM"))
        psP = ctx.enter_context(tc.tile_pool(name="psP", bufs=1, space="PSUM"))
        psW = ctx.enter_context(tc.tile_pool(name="psW", bufs=2, space="PSUM"))

        def load_bias(ap, name):
            t = const.tile([128, 2, 1], F32, name=name)
            src = ap.rearrange("(mt p one) -> mt p one", p=128, one=1)
            for mt in range(2):
                nc.sync.dma_start(out=t[:, mt, :], in_=src[mt])
            return t

        def load_w(src_ap, width, tag, nt=4):
            dst = wpool.tile([128, nt, width], F32R, tag=tag, name=tag)
            nc.sync.dma_start(
                out=dst[:], in_=src_ap.rearrange("(ct p) w -> p ct w", p=128))
            return dst

        def load_ctx(src_ap, width, tag, nsplit):
            dst = ctxp.tile([128, 4, width], F32R, tag=tag, name=tag)
            src = src_ap.rearrange("(ct p) w -> p ct w", p=128)
            step = width // nsplit
            for s in range(nsplit):
                nc.sync.dma_start(out=dst[:, :, s * step:(s + 1) * step],
                                  in_=src[:, :, s * step:(s + 1) * step])
            return dst

        def ctx_tile(width, tag):
            return ctxp.tile([128, 4, width], F32R, tag=tag, name=tag)

        def ctx_chunk(dst, src_ap, width, s, nsplit):
            src = src_ap.rearrange("(ct p) w -> p ct w", p=128)
            step = width // nsplit
            nc.sync.dma_start(out=dst[:, :, s * step:(s + 1) * step],
                              in_=src[:, :, s * step:(s + 1) * step])

        # DMA order: the chunks feeding attend(h0,audio)'s first QK go
        # first; everything later is deliberately DMA-gated so the PE
        # prefers the attend chain once it unblocks.
        xTr = ctx_tile(N, "xTr")
        caTr = ctx_tile(MA, "caTr")
        wqTr = load_w(wqT, HS, "wqTr")
        ctx_chunk(xTr, xT, N, 0, 4)
        wkaTr = load_w(wkaT, HS, "wkaTr")
        ctx_chunk(caTr, caT, MA, 0, 4)
        bq_t = load_bias(bq, "bq_t")
        bka_t = load_bias(bka, "bka_t")
        ctx_chunk(xTr, xT, N, 1, 4)
        wvaTr = load_w(wvaT, HS, "wvaTr")
        ctx_chunk(caTr, caT, MA, 1, 4)
        bks_t = load_bias(bks, "bks_t")
        bvv_t = load_bias(bvv, "bvv_t")
        ctx_chunk(caTr, caT, MA, 2, 4)
        ctx_chunk(caTr, caT, MA, 3, 4)
        ctx_chunk(xTr, xT, N, 2, 4)
        ctx_chunk(xTr, xT, N, 3, 4)
        wksTr = load_w(wksT, HS, "wksTr")
        wvsTr = load_w(wvsT, HS, "wvsTr")
        csTr = load_ctx(csT, MS, "csTr", 1)
        wpTr = load_w(wpT, DIM, "wpTr", nt=2)

        # Persistent activation tiles
        qz = actp.tile([128, 4, N], F32R, tag="qz", name="qz")
        kTa = actp.tile([128, 2, MA], F32R, tag="kTa", name="kTa")
        kTs = actp.tile([128, 2, MS], F32R, tag="kTs", name="kTs")
        vn_a = actp.tile([128, MA // 128, 4, VPAD], F32R, tag="vn_a",
                         name="vn_a")
        vn_s = actp.tile([128, MS // 128, 4, VPAD], F32R, tag="vn_s",
                         name="vn_s")
        zTs = [actp.tile([128, N], F32R, tag=f"zT{mt}", name=f"zT{mt}")
               for mt in range(2)]

        # zero the unused q half per head slot (rows 64:128 for even heads,
        # 0:64 for odd) and set the v ones columns -- all early, off ACT.
        nc.vector.memset(qz[64:128, 0, :], 0.0)
        nc.gpsimd.memset(qz[0:64, 1, :], 0.0)
        nc.vector.memset(qz[64:128, 2, :], 0.0)
        nc.gpsimd.memset(qz[0:64, 3, :], 0.0)
        nc.vector.memset(vn_a[:, :, :, HD:HD + 1], 1.0)
        nc.vector.memset(vn_s[:, :, :, HD:HD + 1], 1.0)

        def q_step(mt, n5):
            """q proj for head pair mt, one 512-col chunk."""
            acc = psW.tile([128, 512], F32, tag="w", name=f"q{mt}_{n5}")
            sl = slice(n5 * 512, (n5 + 1) * 512)
            for ct in range(4):
                nc.tensor.matmul(acc[:], wqTr[:, ct, mt * 128:(mt + 1) * 128],
                                 xTr[:, ct, sl],
                                 start=(ct == 0), stop=(ct == 3))
            h0, h1 = 2 * mt, 2 * mt + 1
            nc.vector.tensor_scalar_add(qz[0:64, h0, sl], acc[0:64, :],
                                        bq_t[0:64, mt, :])
            nc.vector.tensor_scalar_add(qz[64:128, h1, sl], acc[64:128, :],
                                        bq_t[64:128, mt, :])

        def k_step(w_t, src, k_dst, bias, pair, m5, wchunk, tag):
            """k proj for head pair, one wchunk-col chunk of context."""
            acc = psW.tile([128, 512], F32, tag="w", name=f"k{tag}{pair}_{m5}")
            sl = slice(m5 * wchunk, (m5 + 1) * wchunk)
            for ct in range(4):
                nc.tensor.matmul(acc[:, 0:wchunk],
                                 w_t[:, ct, pair * 128:(pair + 1) * 128],
                                 src[:, ct, sl],
                                 start=(ct == 0), stop=(ct == 3))
            nc.vector.tensor_scalar_add(k_dst[:, pair, sl], acc[:, 0:wchunk],
                                        bias[:, pair, :])

        def v_step(w_t, ct_src, vn_dst, m2, tag):
            """v in natural layout for 2 context m-tiles (256 tokens).
            stationary = ctx tile [feat,128tok]; moving = Wv chunk."""
            acc = psW.tile([128, 512], F32, tag="w", name=f"v{tag}{m2}")
            for mi in range(2):
                m_t = m2 * 2 + mi
                for ct in range(4):
                    nc.tensor.matmul(
                        acc[:, mi * 256:(mi + 1) * 256],
                        ct_src[:, ct, m_t * 128:(m_t + 1) * 128],
                        w_t[:, ct, :],
                        start=(ct == 0), stop=(ct == 3))
            nc.vector.tensor_copy(
                vn_dst[:, m2 * 2:m2 * 2 + 2, :, 0:HD],
                acc[:].rearrange("p (mi h d) -> p mi h d", h=4, d=HD))

        # --- prefix: everything attend(h0, audio) reads (emission order
        # defines dependency direction -- a pv_step emitted before the
        # v_step that fills its vn tile would read garbage). The later
        # items are gated by their DMA chunks, so the first QK still
        # executes after only q(0,0..1)+k(0) and the rest of the prefix
        # trickles in under the exp stream.
        q_step(0, 0)
        q_step(0, 1)
        k_step(wkaTr, caTr, kTa, bka_t, 0, 0, 512, "a")
        v_step(wvaTr, caTr, vn_a, 0, "a")
        v_step(wvaTr, caTr, vn_a, 1, "a")
        k_step(wkaTr, caTr, kTa, bka_t, 0, 1, 512, "a")
        v_step(wvaTr, caTr, vn_a, 2, "a")
        v_step(wvaTr, caTr, vn_a, 3, "a")
        k_step(wkaTr, caTr, kTa, bka_t, 0, 2, 512, "a")
        v_step(wvaTr, caTr, vn_a, 4, "a")
        v_step(wvaTr, caTr, vn_a, 5, "a")
        k_step(wkaTr, caTr, kTa, bka_t, 0, 3, 512, "a")
        v_step(wvaTr, caTr, vn_a, 6, "a")
        v_step(wvaTr, caTr, vn_a, 7, "a")

        def attend(h, kT, vn, m_total, nm, mid_hook=None):
            """Full attention for head h; returns (u, rb).
            u rows prow:prow+64 = unnormalized o^T; rb = 1/denominator
            broadcast to all partitions. Runs per query half (nh) so pv
            needs one [65,1024] slot. PV is emitted 2 m-tiles behind QK
            so the in-order PE stream alternates QK(i), PV(i-2): the PV
            deps are then long-satisfied and never stall the stream that
            feeds the (bottleneck) scalar-engine exp pipeline."""
            pair = h // 2
            prow = (h % 2) * 64
            mts = m_total // 128
            q_h = qz[:, h, :]
            u = up.tile([128, N], F32, tag="u", name=f"u{h}{nm}")
            rb = rpool.tile([128, N], F32, tag="rb", name=f"rb{h}{nm}")
            npairs = mts // 2
            for nh in range(2):
                nsl = slice(nh * MMN, (nh + 1) * MMN)
                pv = psP.tile([65, MMN], F32, tag="pv", name=f"pv{h}{nm}{nh}")
                e2s = {}

                def qk_exp(m_t):
                    if m_t % 2 == 0:
                        e2s[m_t // 2] = ep.tile([128, 2, MMN], F32R, tag="eT",
                                                name=f"e{h}{nm}{nh}_{m_t // 2}")
                    e2 = e2s[m_t // 2]
                    sA = psS.tile([128, MMN], F32, tag="sA",
                                  name=f"sA{h}{nm}{nh}_{m_t}")
                    for j in range(2):
                        nc.tensor.matmul(
                            sA[:, j * 512:(j + 1) * 512],
                            kT[:, pair, m_t * 128:(m_t + 1) * 128],
                            q_h[:, nh * MMN + j * 512:nh * MMN + (j + 1) * 512],
                            start=True, stop=True)
                    nc.scalar.activation(e2[:, m_t % 2, :], sA[:],
                                         AF.Exp, scale=SCALE)

                def pv_step(m_t):
                    e2 = e2s[m_t // 2]
                    lhs_v = vn[:, m_t, h, 0:HD + 1]
                    for j in range(2):
                        nc.tensor.matmul(
                            pv[:, j * 512:(j + 1) * 512], lhs_v,
                            e2[:, m_t % 2, j * 512:(j + 1) * 512],
                            start=(m_t == 0), stop=(m_t == mts - 1))

                lag = 8 if mts >= 8 else 2
                for m_t in range(min(lag, mts)):
                    qk_exp(m_t)
                for m_t in range(lag, mts):
                    qk_exp(m_t)
                    pv_step(m_t - lag)
                for m_t in range(max(0, mts - lag), mts):
                    pv_step(m_t)

                nc.vector.tensor_copy(u[prow:prow + 64, nsl], pv[0:64, :])
                nc.vector.tensor_copy(rb[0:1, nsl], pv[64:65, :])
                nc.vector.reciprocal_approx_fast(rb[0:1, nsl], rb[0:1, nsl])
                nc.gpsimd.partition_broadcast(rb[:, nsl], rb[0:1, nsl])
                if nh == 0 and mid_hook is not None:
                    mid_hook()
            return u, rb

        def h0_mid():
            # emitted between attend(h0,audio) halves: nh=1 needs q cols
            # 1024:2048 (first read there, so this ordering is safe).
            q_step(0, 2)
            q_step(0, 3)

        def ft0_chunk(n5):
            # zT0 out-proj half (heads 0,1): runs in PE/DVE slack under
            # heads 2-3's exp stream; ships as its own fp16 partial.
            for ot in range(4):
                acc = psW.tile([128, 512], F32, tag="w",
                               name=f"f0_{n5}_{ot}")
                nc.tensor.matmul(
                    acc[:], wpTr[:, 0, ot * 128:(ot + 1) * 128],
                    zTs[0][:, n5 * 512:(n5 + 1) * 512],
                    start=True, stop=True)
                ob = ostage.tile([128, 512], F16, tag="ob",
                                 name=f"oba{n5}_{ot}")
                nc.vector.tensor_copy(ob[:], acc[:])
                nc.sync.dma_start(
                    out=out_a[ot * 128:(ot + 1) * 128,
                              n5 * 512:(n5 + 1) * 512],
                    in_=ob[:])

        # --- attends, with remaining projections emitted into the slack --
        for h in range(4):
            prow = (h % 2) * 64
            z_h = zTs[h // 2][prow:prow + 64, :]
            tmp = rpool.tile([128, N], F32, tag="rb", name=f"tmp{h}")

            if h < 3:
                u_a, rb_a = attend(h, kTa, vn_a, MA, "a",
                                   mid_hook=h0_mid if h == 0 else None)
                if h == 2:
                    ft0_chunk(0)
                    ft0_chunk(1)
                if h == 0:
                    # rest of the projections: fill PE slack under the
                    # audio exp stream of head 0.
                    for pair in range(2):
                        k_step(wksTr, csTr, kTs, bks_t, pair, 0, 256, "s")
                    v_step(wvsTr, csTr, vn_s, 0, "s")
                    for n5 in range(4):
                        q_step(1, n5)
                    for m5 in range(4):
                        k_step(wkaTr, caTr, kTa, bka_t, 1, m5, 512, "a")
                u_s, rb_s = attend(h, kTs, vn_s, MS, "s")
            else:
                # last head: singer first so the tail chain is audio-only.
                u_s, rb_s = attend(h, kTs, vn_s, MS, "s")
                for ni in range(4):
                    sl = slice(ni * 512, (ni + 1) * 512)
                    nc.vector.tensor_tensor(
                        tmp[prow:prow + 64, sl], u_s[prow:prow + 64, sl],
                        rb_s[prow:prow + 64, sl], op=OP.mult)
                u_a, rb_a = attend(h, kTa, vn_a, MA, "a")

            for ni in range(4):
                sl = slice(ni * 512, (ni + 1) * 512)
                nc.vector.tensor_tensor(
                    z_h[:, sl], u_a[prow:prow + 64, sl],
                    rb_a[prow:prow + 64, sl], op=OP.mult)
            for ni in range(4):
                sl = slice(ni * 512, (ni + 1) * 512)
                if h < 3:
                    nc.vector.tensor_tensor(
                        tmp[prow:prow + 64, sl], u_s[prow:prow + 64, sl],
                        rb_s[prow:prow + 64, sl], op=OP.mult)
                # z = (tmp + bvv) + z
                nc.vector.scalar_tensor_tensor(
                    z_h[:, sl], tmp[prow:prow + 64, sl],
                    bvv_t[prow:prow + 64, h // 2, :],
                    z_h[:, sl], op0=OP.add, op1=OP.add)

            if h == 2:
                ft0_chunk(2)
                ft0_chunk(3)


        # --- output projection tail: the zT1 contraction half only (zT0
        # was projected mid-stream after combine(h1)); host sums partials.
        for n5 in range(4):
            for ot in range(4):
                acc = psW.tile([128, 512], F32, tag="w", name=f"f1_{n5}_{ot}")
                nc.tensor.matmul(acc[:], wpTr[:, 1, ot * 128:(ot + 1) * 128],
                                 zTs[1][:, n5 * 512:(n5 + 1) * 512],
                                 start=True, stop=True)
                ob = ostage.tile([128, 512], F16, tag="ob",
                                 name=f"obb{n5}_{ot}")
                if (n5 + ot) % 2:
                    nc.scalar.copy(ob[:], acc[:])
                else:
                    nc.vector.tensor_copy(ob[:], acc[:])
                nc.sync.dma_start(
                    out=out_b[ot * 128:(ot + 1) * 128,
                              n5 * 512:(n5 + 1) * 512],
                    in_=ob[:])

    nc.compile()
    return nc


_CACHE = {}


def _get_nc():
    if "nc" not in _CACHE:
        _CACHE["nc"] = _build()
    return _CACHE["nc"]


def _make_in_maps(inputs):
    x = np.asarray(inputs["x"], np.float32)
    ca = np.asarray(inputs["audio_context"], np.float32)
    cs = np.asarray(inputs["singer_context"], np.float32)
    W = {k: np.asarray(inputs[k], np.float32)
         for k in ("Wq", "Wka", "Wva", "Wks", "Wvs", "Wp")}
    bias = {k: np.asarray(inputs[k], np.float32)
            for k in ("bq", "bka", "bva", "bks", "bvs", "bp")}

    c = np.ascontiguousarray

    def cb(a):  # contiguous bf16
        return np.ascontiguousarray(a).astype(ml_dtypes.bfloat16)

    in_maps = []
    for core in range(8):
        bi, hg = core // 2, core % 2
        hs = slice(hg * HS, (hg + 1) * HS)
        in_maps.append({
            "xT": cb(x[bi].T),
            "caT": cb(ca[bi].T),
            "csT": cb(cs[bi].T),
            "wqT": cb(W["Wq"][hs, :].T),
            "wkaT": cb(W["Wka"][hs, :].T),
            "wvaT": cb(W["Wva"][hs, :].T),
            "wksT": cb(W["Wks"][hs, :].T),
            "wvsT": cb(W["Wvs"][hs, :].T),
            "wpT": cb(W["Wp"][:, hs].T),
            "bq": c(bias["bq"][hs]),
            "bka": c(bias["bka"][hs]),
            "bks": c(bias["bks"][hs]),
            "bvv": c(bias["bva"][hs] + bias["bvs"][hs]),
        })
    return in_maps


def kernel(**inputs) -> np.ndarray:
    nc = _get_nc()
    in_maps = _make_in_maps(inputs)
    res = bass_utils.run_bass_kernel_spmd(nc, in_maps, core_ids=list(range(8)))
    bp = np.asarray(inputs["bp"], np.float32)
    out = np.empty((B, N, DIM), np.float32)
    for bi in range(B):
        s = (res.results[2 * bi]["out_a"].astype(np.float32)
             + res.results[2 * bi]["out_b"].astype(np.float32)
             + res.results[2 * bi + 1]["out_a"].astype(np.float32)
             + res.results[2 * bi + 1]["out_b"].astype(np.float32))
        out[bi] = s.T + bp
    return out
